# revision 21
# baseline (speedup 1.0000x reference)
"""Trainium2 Bass kernel for a single attention layer (Baichuan-style W_pack
attention with rotary embeddings), sharded over 8 NeuronCores:
tensor-parallel over 4 head groups x data-parallel over 2 batches.

bf16 matmul operands (f32 PSUM accumulate), q/at SBUF-resident, k/v via bf16
DRAM scratch, exact causal block trims with a corner-triangle mask, softmax
denominator broadcast via a ones[128,128] stationary matmul, and the output
projection split into two chunks pipelined between attention phases.

Contract: kernel(**inputs) takes the FULL unsharded inputs and returns the
FULL output [2, 2048, 4096] float32. All sharding / gathering happens here.
"""

import math
import sys

import numpy as np

for _p in ("/opt/trn_rl_repo", "/root/.axon_site/_ro/trn_rl_repo"):
    if _p not in sys.path:
        sys.path.insert(0, _p)

import ml_dtypes

BF16_NP = ml_dtypes.bfloat16

HIDDEN = 4096
N_HEADS = 32
HEAD_DIM = 128
BASE = 10000.0
B = 2
S = 2048
HEADS_PER_CORE = 8          # 32 heads / 4 groups
HG = 1024                   # head-group width = 8 heads * 128
NEG_BIG = -1.0e9

# RoPE partner permutation: quadrant q holds [lo_d 16q..16q+15, hi_d 64+16q..]
# so the rotate-half partner of new-row i is i+-16 inside its 32-row quadrant,
# reachable by DVE stream_shuffle.
PERM = np.zeros(128, dtype=np.int64)
for _q in range(4):
    PERM[32 * _q: 32 * _q + 16] = np.arange(16 * _q, 16 * _q + 16)
    PERM[32 * _q + 16: 32 * _q + 32] = 64 + np.arange(16 * _q, 16 * _q + 16)
SHUF_MASK = [(i + 16) % 32 for i in range(32)]
# sign of the sin term per (new) row: -1 where original d < 64
SIGN = np.where(PERM < 64, -1.0, 1.0).astype(np.float32)


def _rope_tables(max_pos):
    inv_freq = 1.0 / (BASE ** (np.arange(0, HEAD_DIM, 2, dtype=np.float32) / HEAD_DIM))
    t = np.arange(max_pos, dtype=np.float32)
    freqs = np.outer(t, inv_freq)                      # [P, 64]
    emb = np.concatenate((freqs, freqs), axis=-1)      # [P, 128]
    return np.cos(emb).astype(np.float32), np.sin(emb).astype(np.float32)


def _build_program(mask_mode):
    """mask_mode: 'causal' (block-skip + corner triangle mask),
    'none' (dense, no mask), 'full' (dense, stream mask tiles)."""
    import concourse.bacc as bacc
    import concourse.mybir as mybir
    import concourse.tile as tile
    from contextlib import ExitStack

    F32 = mybir.dt.float32
    BF16 = mybir.dt.bfloat16
    ALU = mybir.AluOpType
    ACTF = mybir.ActivationFunctionType

    nc = bacc.Bacc("TRN2", target_bir_lowering=False, debug=False)

    xT = nc.declare_dram_parameter("xT", [HIDDEN, S], BF16, isOutput=False)
    wqkT = nc.declare_dram_parameter("wqkT", [HIDDEN, 2 * HG], BF16, isOutput=False)
    wvT = nc.declare_dram_parameter("wvT", [HIDDEN, HG], BF16, isOutput=False)
    woT = nc.declare_dram_parameter("woT", [HG, HIDDEN], BF16, isOutput=False)
    cosT = nc.declare_dram_parameter("cosT", [128, S], F32, isOutput=False)
    sinT = nc.declare_dram_parameter("sinT", [128, S], F32, isOutput=False)
    if mask_mode == "causal":
        cmask = nc.declare_dram_parameter("cmask", [128, 128], F32, isOutput=False)
    elif mask_mode == "full":
        maskT = nc.declare_dram_parameter("maskT", [S, S], F32, isOutput=False)
    out_p = nc.declare_dram_parameter("out_p", [S, HIDDEN], F32, isOutput=True)

    kT_s = nc.dram_tensor("kT_scratch", [HG, S], BF16)
    v_s = nc.dram_tensor("v_scratch", [S, HG], BF16)

    inv_sqrt_d = 1.0 / math.sqrt(HEAD_DIM)
    causal = mask_mode == "causal"

    # h-chunk sweep order: B-block (16..31) first, A-block (0..15) last, so
    # the A pool's refill for the next half overlaps B-block compute
    H_ORDER = list(range(16, 32)) + list(range(16))

    with tile.TileContext(nc, pool_alloc_mode="queue") as tc, ExitStack() as top:
        const_pool = top.enter_context(tc.tile_pool(name="consts", bufs=1))
        ones_f32 = const_pool.tile([128, 128], F32)
        nc.vector.memset(ones_f32, 1.0)
        ones128 = const_pool.tile([128, 128], BF16)
        nc.vector.tensor_copy(ones128, ones_f32)
        if causal:
            cmask_sb = const_pool.tile([128, 128], F32)
            nc.sync.dma_start(out=cmask_sb, in_=cmask.ap())

        # q and at stay SBUF-resident (no DRAM round trip)
        q_pool = top.enter_context(tc.tile_pool(name="qsb", bufs=1))
        q_sb = q_pool.tile([128, HEADS_PER_CORE, S], BF16)
        at_pool = top.enter_context(tc.tile_pool(name="atsb", bufs=3))
        mpool = None
        if mask_mode == "full":
            mpool = top.enter_context(tc.tile_pool(name="msk", bufs=3))
        # fixed-size pools shared across halves/chunks (keeps the queue-mode
        # ring allocator from fragmenting)
        wpool = top.enter_context(tc.tile_pool(name="wqk", bufs=2))
        rpool = top.enter_context(tc.tile_pool(name="rope", bufs=2))
        kstg = top.enter_context(tc.tile_pool(name="kstg", bufs=2))
        wvp = top.enter_context(tc.tile_pool(name="wvt", bufs=2))
        voutp = top.enter_context(tc.tile_pool(name="vout", bufs=4))
        wop = top.enter_context(tc.tile_pool(name="wo", bufs=2))
        otp = top.enter_context(tc.tile_pool(name="ot", bufs=2))

        # ---------------- Phase A: QKV projection (+RoPE on q,k) -------------
        def load_x_half(stack, hs):
            s0 = hs * 1024
            xpoolA = stack.enter_context(tc.tile_pool(name=f"xh{hs}A", bufs=1))
            xpoolB = stack.enter_context(tc.tile_pool(name=f"xh{hs}B", bufs=1))
            xtA = xpoolA.tile([128, 16, 1024], BF16, name=f"xtA{hs}")
            xtB = xpoolB.tile([128, 16, 1024], BF16, name=f"xtB{hs}")
            xin = xT.ap()[:, s0:s0 + 1024].rearrange("(c p) s -> p c s", p=128)
            for c in range(16, 32):
                nc.sync.dma_start(out=xtB[:, c - 16, :], in_=xin[:, c, :])
            for c in range(16):
                nc.sync.dma_start(out=xtA[:, c, :], in_=xin[:, c, :])
            cspool = stack.enter_context(tc.tile_pool(name=f"cs{hs}", bufs=1))
            cos_sb = cspool.tile([128, 1024], F32)
            nc.sync.dma_start(out=cos_sb, in_=cosT.ap()[:, s0:s0 + 1024])
            sin_sb = cspool.tile([128, 1024], F32)
            nc.sync.dma_start(out=sin_sb, in_=sinT.ap()[:, s0:s0 + 1024])

            def xt_slice(c, sl):
                return xtB[:, c - 16, sl] if c >= 16 else xtA[:, c, sl]

            return xt_slice, cos_sb, sin_sb

        def fetch_wqk(oc):
            w_oc = wpool.tile([128, 32, 128], BF16, tag="w_oc")
            w_in = wqkT.ap()[:, oc * 128:(oc + 1) * 128].rearrange(
                "(c p) o -> p c o", p=128)
            nc.sync.dma_start(out=w_oc, in_=w_in)
            return w_oc

        def emit_proj_qk(hs, xt_slice, cos_sb, sin_sb, w0=None):
            s0 = hs * 1024
            with ExitStack() as qk:
                pqk = qk.enter_context(
                    tc.tile_pool(name="pqk", bufs=2, space="PSUM"))
                for oc in range(16):         # o chunks of 128 (head tiles)
                    w_oc = w0 if (oc == 0 and w0 is not None) else fetch_wqk(oc)
                    pk = pqk.tile([128, 2, 512], F32, tag="pk")
                    for hi, h in enumerate(H_ORDER):
                        for sc in range(2):
                            nc.tensor.matmul(
                                pk[:, sc, :], w_oc[:, h, :],
                                xt_slice(h, slice(sc * 512, (sc + 1) * 512)),
                                start=(hi == 0), stop=(hi == 31))
                    # RoPE: q' = q*cos + shuffle16(q)*sin_signed
                    hh = oc % 8
                    for sc in range(2):
                        cs = cos_sb[:, sc * 512:(sc + 1) * 512]
                        sn = sin_sb[:, sc * 512:(sc + 1) * 512]
                        qrot = rpool.tile([128, 512], F32, tag="qrot")
                        nc.vector.stream_shuffle(qrot, pk[:, sc, :], SHUF_MASK)
                        t1 = rpool.tile([128, 512], F32, tag="t1")
                        nc.vector.tensor_tensor(t1, pk[:, sc, :], cs, ALU.mult)
                        t2 = rpool.tile([128, 512], F32, tag="t2")
                        nc.gpsimd.tensor_tensor(t2, qrot, sn, ALU.mult)
                        if oc < 8:
                            # q written straight into resident SBUF
                            nc.vector.tensor_tensor(
                                q_sb[:, oc, s0 + sc * 512: s0 + (sc + 1) * 512],
                                t1, t2, ALU.add)
                        else:
                            ks = kstg.tile([128, 512], BF16, tag="ks")
                            nc.vector.tensor_tensor(ks, t1, t2, ALU.add)
                            nc.scalar.dma_start(
                                out=kT_s.ap()[hh * 128:(hh + 1) * 128,
                                              s0 + sc * 512: s0 + (sc + 1) * 512],
                                in_=ks)

        H_GROUPS = [list(range(16, 24)), list(range(24, 32)),
                    list(range(0, 8)), list(range(8, 16))]

        def emit_proj_v(hs, xt_slice):
            s0 = hs * 1024
            with ExitStack() as vv:
                pv = vv.enter_context(
                    tc.tile_pool(name="pv", bufs=4, space="PSUM"))
                vout = voutp
                def fetch_wv(ov, gi):
                    grp = H_GROUPS[gi]
                    wv_g = wvp.tile([128, 8, 512], BF16, tag="wv_g")
                    nc.sync.dma_start(
                        out=wv_g,
                        in_=wvT.ap()[grp[0] * 128:(grp[-1] + 1) * 128,
                                     ov * 512:(ov + 1) * 512].rearrange(
                            "(c p) o -> p c o", p=128))
                    return wv_g

                # scg0 first for both ov chunks: the next attention phase's
                # first vtile gathers only need the low s rows
                steps = [(ov, scg, gi) for scg in range(2) for ov in range(2)
                         for gi in range(4)]
                wv_next = fetch_wv(*[(s[0], s[2]) for s in steps][0])
                for si, (ov, scg, gi) in enumerate(steps):
                    if gi == 0:
                        vb = [pv.tile([128, 512], F32, tag="vb",
                                      name=f"vb{hs}{ov}{scg}{i}")
                              for i in range(4)]
                    wv_g = wv_next
                    if si + 1 < len(steps):
                        nxt = steps[si + 1]
                        wv_next = fetch_wv(nxt[0], nxt[2])
                    for ih, h in enumerate(H_GROUPS[gi]):
                        for s4 in range(4):
                            sc = scg * 4 + s4
                            nc.tensor.matmul(
                                vb[s4],
                                xt_slice(h, slice(sc * 128, (sc + 1) * 128)),
                                wv_g[:, ih, :],
                                start=(gi == 0 and ih == 0),
                                stop=(gi == 3 and ih == 7))
                    if gi == 3:
                        for s4 in range(4):
                            sc = scg * 4 + s4
                            vo = vout.tile([128, 512], BF16, tag="vo")
                            nc.vector.tensor_copy(vo, vb[s4])
                            nc.scalar.dma_start(
                                out=v_s.ap()[s0 + sc * 128: s0 + (sc + 1) * 128,
                                             ov * 512:(ov + 1) * 512],
                                in_=vo)

        # ---------------- Phase B: attention for one 512-wide q chunk --------
        def emit_attn_qc(qc, at_tile):
            """Per head: scores+EXP for all blocks first (off-diag pairs share
            one EXP; diag blocks trimmed + corner mask), then av/den matmuls.
            For qc==0 the av/den half is deferred by one head so the EXP
            latency hides behind the next head's score matmuls."""
            nblk = 4 * qc + 4 if causal else 16
            n_off = 4 * qc if causal else nblk
            defer = causal and qc == 0
            with ExitStack() as phb:
                kvp = phb.enter_context(tc.tile_pool(name="kv", bufs=3))
                esp = phb.enter_context(tc.tile_pool(name="es", bufs=8))
                smallp = phb.enter_context(tc.tile_pool(name="small", bufs=2))
                ps = phb.enter_context(
                    tc.tile_pool(name="ps", bufs=3, space="PSUM"))
                pav = phb.enter_context(
                    tc.tile_pool(name="pav", bufs=1, space="PSUM"))
                pden = phb.enter_context(
                    tc.tile_pool(name="pden", bufs=1, space="PSUM"))

                def emit_avden(hh, vtile, es_info):
                    av = pav.tile([128, 512], F32, tag="av")
                    den = pden.tile([128, 512], F32, tag="den")
                    for kb in range(nblk):
                        es_sl, q_lo = es_info[kb]
                        qs = slice(q_lo, 512)
                        last = kb == nblk - 1
                        nc.tensor.matmul(
                            av[:, qs], vtile[:, kb, :], es_sl,
                            start=(kb == 0), stop=last,
                            skip_group_check=causal)
                        nc.tensor.matmul(
                            den[:, qs], ones128[:, :], es_sl,
                            start=(kb == 0), stop=last,
                            skip_group_check=causal)
                    recip = smallp.tile([128, 512], F32, tag="recip")
                    nc.vector.reciprocal_approx_fast(recip, den)
                    nc.vector.tensor_tensor(
                        at_tile[:, hh, :], av, recip, ALU.mult)

                pending = None
                for hh in range(HEADS_PER_CORE):
                    ktile = kvp.tile([128, nblk * 128], BF16, tag="ktile",
                                     name=f"kt{qc}_{hh}")
                    nc.sync.dma_start(
                        out=ktile,
                        in_=kT_s.ap()[hh * 128:(hh + 1) * 128, 0:nblk * 128])
                    vtile = kvp.tile([128, nblk, 128], BF16, tag="vtile",
                                     name=f"vt{qc}_{hh}")
                    nc.sync.dma_start(
                        out=vtile,
                        in_=v_s.ap()[0:nblk * 128,
                                     hh * 128:(hh + 1) * 128].rearrange(
                            "(b p) d -> p b d", p=128))
                    qmv = q_sb[:, hh, qc * 512:(qc + 1) * 512]
                    es_info = [None] * nblk
                    # off-diagonal pairs: two scores, one EXP over both banks
                    for p in range(n_off // 2):
                        sps = ps.tile([128, 2, 512], F32, tag="sps")
                        es = esp.tile([128, 2, 512], BF16, tag="es")
                        for j in (0, 1):
                            kb = 2 * p + j
                            nc.tensor.matmul(
                                sps[:, j, :],
                                ktile[:, kb * 128:(kb + 1) * 128], qmv,
                                start=True, stop=True)
                            if mask_mode == "full":
                                mt = mpool.tile([128, 512], F32, tag="mt")
                                nc.sync.dma_start(
                                    out=mt,
                                    in_=maskT.ap()[kb * 128:(kb + 1) * 128,
                                                   qc * 512:(qc + 1) * 512])
                                nc.vector.tensor_tensor(
                                    sps[:, j, :], sps[:, j, :], mt, ALU.add)
                            es_info[kb] = (es[:, j, :], 0)
                        nc.scalar.activation(es[:, :, :], sps[:, :, :],
                                             ACTF.Exp, scale=inv_sqrt_d)
                    # diagonal blocks: trimmed scores + corner mask + EXP
                    if causal:
                        for dp in range(2):
                            sps = ps.tile([128, 2, 512], F32, tag="sps")
                            es = esp.tile([128, 2, 512], BF16, tag="es")
                            for j in (0, 1):
                                v = 2 * dp + j
                                kb = 4 * qc + v
                                q_lo = 128 * v
                                qs = slice(q_lo, 512)
                                nc.tensor.matmul(
                                    sps[:, j, qs],
                                    ktile[:, kb * 128:(kb + 1) * 128],
                                    qmv[:, qs], start=True, stop=True)
                                nc.vector.tensor_tensor(
                                    sps[:, j, q_lo:q_lo + 128],
                                    sps[:, j, q_lo:q_lo + 128],
                                    cmask_sb, ALU.add)
                                nc.scalar.activation(
                                    es[:, j, qs], sps[:, j, qs],
                                    ACTF.Exp, scale=inv_sqrt_d)
                                es_info[kb] = (es[:, j, qs], q_lo)
                    if defer:
                        if pending is not None:
                            emit_avden(*pending)
                        pending = (hh, vtile, es_info)
                    else:
                        emit_avden(hh, vtile, es_info)
                if pending is not None:
                    emit_avden(*pending)

        # ---------------- Phase C: output projection for 8 s-tiles -----------
        def fetch_wo(oc):
            wo_sl = wop.tile([128, 8, 512], BF16, tag="wo_sl")
            wo_in = woT.ap()[:, oc * 512:(oc + 1) * 512].rearrange(
                "(c p) o -> p c o", p=128)
            nc.scalar.dma_start(out=wo_sl, in_=wo_in)
            return wo_sl

        def emit_oproj(chunk, at_tiles, wo0=None):
            # chunk 0 -> st 0..7 (at tiles for qc 0,1); chunk 1 -> st 8..15
            with ExitStack() as phc:
                pop = phc.enter_context(
                    tc.tile_pool(name="pop", bufs=2, space="PSUM"))
                wo_next = wo0 if wo0 is not None else fetch_wo(0)
                for oc in range(8):              # output chunks of 512
                    wo_sl = wo_next
                    if oc + 1 < 8:
                        wo_next = fetch_wo(oc + 1)
                    for sl in range(8):          # s tiles of 128 within chunk
                        st = chunk * 8 + sl
                        at_t = at_tiles[sl // 4]
                        ats = at_t[:, :, (sl % 4) * 128:(sl % 4 + 1) * 128]
                        op = pop.tile([128, 512], F32, tag="op")
                        for hc in range(8):
                            nc.tensor.matmul(
                                op, ats[:, hc, :], wo_sl[:, hc, :],
                                start=(hc == 0), stop=(hc == 7))
                        ot = otp.tile([128, 512], F32, tag="ot")
                        nc.scalar.copy(ot, op)
                        nc.gpsimd.dma_start(
                            out=out_p.ap()[st * 128:(st + 1) * 128,
                                           oc * 512:(oc + 1) * 512],
                            in_=ot)

        # ---------------- schedule -------------------------------------------
        def new_at(qc):
            return at_pool.tile([128, HEADS_PER_CORE, 512], BF16,
                                tag="at", name=f"at{qc}")

        at_tiles = {}
        w0_h0 = fetch_wqk(0)        # first qk weight beats the x stream
        with ExitStack() as h0:
            xt0, cos0, sin0 = load_x_half(h0, 0)
            emit_proj_qk(0, xt0, cos0, sin0, w0=w0_h0)
            emit_proj_v(0, xt0)
        if causal:
            for qc in (0, 1):
                at_tiles[qc] = new_at(qc)
                emit_attn_qc(qc, at_tiles[qc])
        wo0_c0 = fetch_wo(0) if causal else None
        with ExitStack() as h1:
            xt1, cos1, sin1 = load_x_half(h1, 1)
            if causal:
                emit_oproj(0, [at_tiles[0], at_tiles[1]], wo0=wo0_c0)
            emit_proj_qk(1, xt1, cos1, sin1)
            emit_proj_v(1, xt1)
        for qc in (2, 3) if causal else (0, 1, 2, 3):
            if causal and qc == 3:
                wo0_c1 = fetch_wo(0)
            at_tiles[qc] = new_at(qc)
            emit_attn_qc(qc, at_tiles[qc])
        if not causal:
            emit_oproj(0, [at_tiles[0], at_tiles[1]])
            wo0_c1 = fetch_wo(0)
        emit_oproj(1, [at_tiles[2], at_tiles[3]], wo0=wo0_c1)

    nc.compile()
    return nc


_PROGRAM_CACHE = {}


def _get_program(mask_mode):
    if mask_mode not in _PROGRAM_CACHE:
        _PROGRAM_CACHE[mask_mode] = _build_program(mask_mode)
    return _PROGRAM_CACHE[mask_mode]


def _classify_mask(attention_mask):
    m = np.asarray(attention_mask)
    if not np.any(m):
        return "none"
    neg = np.float32(np.finfo(np.float32).min)
    causal = np.triu(np.full((S, S), neg, dtype=np.float32), k=1)
    for b in range(m.shape[0]):
        if not np.array_equal(m[b, 0], causal):
            return "full"
    return "causal"


def _prep_core_inputs(hidden_states, attention_mask, position_ids, W_pack, W_o,
                      mask_mode):
    hidden_states = np.asarray(hidden_states, dtype=np.float32)
    W_pack = np.asarray(W_pack, dtype=np.float32)
    W_o = np.asarray(W_o, dtype=np.float32)
    pos = np.asarray(position_ids).astype(np.int64)

    cos_t, sin_t = _rope_tables(int(pos.max()) + 1)
    # per-batch gathered + transposed + row-permuted (+ sign folded into sin)
    cosT_b, sinT_b = [], []
    for b in range(B):
        c = cos_t[pos[b]][:, PERM].T.copy()              # [128, S]
        s = (sin_t[pos[b]][:, PERM] * SIGN[None, :]).T.copy()
        cosT_b.append(np.ascontiguousarray(c))
        sinT_b.append(np.ascontiguousarray(s))

    xT_b = [np.ascontiguousarray(hidden_states[b].T.astype(BF16_NP))
            for b in range(B)]

    cmask_np = None
    maskT_b = None
    if mask_mode == "causal":
        kk = np.arange(128)[:, None]
        qq = np.arange(128)[None, :]
        cmask_np = np.where(kk <= qq, 0.0, NEG_BIG).astype(np.float32)
    elif mask_mode == "full":
        m = np.asarray(attention_mask, dtype=np.float32)
        maskT_b = [np.ascontiguousarray(m[b, 0].T) for b in range(B)]

    in_maps = []
    for c in range(8):
        b, g = c // 4, c % 4
        # per-head d-permuted q/k weight rows, head-major columns in wqkT
        qrows = np.concatenate(
            [g * HG + hh * 128 + PERM for hh in range(HEADS_PER_CORE)])
        krows = HIDDEN + qrows
        vrows = 2 * HIDDEN + g * HG + np.arange(HG)
        wqkT = np.ascontiguousarray(
            np.concatenate([W_pack[qrows], W_pack[krows]], axis=0).T
            .astype(BF16_NP))
        wvT = np.ascontiguousarray(W_pack[vrows].T.astype(BF16_NP))
        woT = np.ascontiguousarray(W_o[:, g * HG:(g + 1) * HG].T.astype(BF16_NP))
        im = {"xT": xT_b[b], "wqkT": wqkT, "wvT": wvT, "woT": woT,
              "cosT": cosT_b[b], "sinT": sinT_b[b]}
        if mask_mode == "causal":
            im["cmask"] = cmask_np
        elif mask_mode == "full":
            im["maskT"] = maskT_b[b]
        in_maps.append(im)
    return in_maps


def _run(hidden_states, attention_mask, position_ids, W_pack, W_o,
         trace=False, trace_kwargs=None):
    from concourse.bass_utils import run_bass_kernel_spmd

    mask_mode = _classify_mask(attention_mask)
    nc = _get_program(mask_mode)
    in_maps = _prep_core_inputs(hidden_states, attention_mask, position_ids,
                                W_pack, W_o, mask_mode)
    try:
        res = run_bass_kernel_spmd(nc, in_maps, list(range(8)), trace=trace,
                                   **(trace_kwargs or {}))
    except Exception:
        # transient NRT_EXEC_UNIT_UNRECOVERABLE wedges recover on retry
        import time as _time
        _time.sleep(15)
        res = run_bass_kernel_spmd(nc, in_maps, list(range(8)), trace=trace,
                                   **(trace_kwargs or {}))
    out = np.zeros((B, S, HIDDEN), dtype=np.float32)
    for c in range(8):
        out[c // 4] += res.results[c]["out_p"]
    return out, res


def kernel(hidden_states, attention_mask, position_ids, W_pack, W_o):
    out, _ = _run(hidden_states, attention_mask, position_ids, W_pack, W_o)
    return out
